# revision 7
# baseline (speedup 1.0000x reference)
"""Trainium2 Bass kernel for ContrastiveAffinityLossWithMemoryV2.

Math: with MARGIN=4 and d = ||a-b|| <= 2 for unit vectors, relu(M-d) = M-d,
so every pairwise term simplifies:
    t*d^2 + (1-t)*(M-d)^2 = d^2 + (1-t)*(16 - 8*d)
The Sum(d^2) and Sum(1-t) parts are *linear* and computed exactly on host from
vector sums.  The only term that needs the full B x B / B x C planes is
    P3 = Sum 8*d * (1-t)
which is what the device computes, sharded over 8 NeuronCores (512 batch rows
per core):
  - PE: u = 2 - 2*S directly (operands pre-scaled by -2 plus an appended
    ones-row contributing the constant 2), bf16 in, fp32 PSUM out
  - ScalarE: d8 = sqrt(64*u + 1e-2) (PSUM -> SBUF, bf16)
  - VectorE: tensor_tensor_reduce: per-partition Sum d8 * mask, where the mask
    (1-t) ships from host with validity/initialization/diagonal masking and
    the pair-orientation fix (t symmetrized with upper-triangle orientation)
    already folded in.
Host combines per-core partials with the closed-form linear terms.
"""

import numpy as np
import ml_dtypes

N_CLASSES = 8192
B = 4096
D = 192  # 256 * 0.75
NCORES = 8
ROWS = B // NCORES  # 512
KDIM = D + 1  # 193: extra contraction row supplies the constant "+2"
CP = 4096  # padded compacted-class count (>= #initialized classes, mult of 512)
MARGIN = 4.0
MEMORY_WEIGHT = 0.5
WARMUP_STEPS = 1000
MOM_WARMUP = 5000
BASE_MOM = 0.9
BG_SIM = 0.2
BG_OTHER_SIM = 0.01
EPS = 1e-12
DELTA2 = 0.01  # bias inside sqrt(64*u + DELTA2); keeps the arg positive

bf16 = ml_dtypes.bfloat16

_CACHE = {}


def cap_bf16(v):
    """fp32 -> bf16 by truncation toward zero, so each row's L2 norm can only
    shrink; guarantees u = 2-2S >= -(fp32 accum noise) on device."""
    f32 = np.ascontiguousarray(v, dtype=np.float32)
    return (f32.view(np.uint32) >> 16).astype(np.uint16).view(bf16)


def _bank_chains(zn, y_true, momentum):
    """Replicate the reference's sequential per-sample EMA scatter (fp32)."""
    valid = (y_true >= 0) & (y_true < N_CLASSES)
    lc = np.clip(y_true, 0, N_CLASSES - 1)
    m = np.float32(momentum)
    one_m = np.float32(1.0 - momentum)
    bank = {}
    for i in np.nonzero(valid)[0]:
        c = int(lc[i])
        if c not in bank:
            bank[c] = zn[i].copy()
        else:
            ema = m * bank[c] + one_m * zn[i]
            n = np.float32(np.sqrt(np.float32((ema ** 2).sum())))
            bank[c] = ema / max(n, np.float32(EPS))
    return bank


def _build_nc():
    from concourse import bass, bacc, tile, mybir

    dt = mybir.dt
    nc = bacc.Bacc("TRN2", target_bir_lowering=False, debug=False)

    lhs_d = nc.dram_tensor("lhs", (KDIM, ROWS), dt.bfloat16, kind="ExternalInput")
    rhs_s_d = nc.dram_tensor("rhs_s", (KDIM, CP), dt.bfloat16, kind="ExternalInput")
    rhs_g_d = nc.dram_tensor("rhs_g", (KDIM, B), dt.bfloat16, kind="ExternalInput")
    r1_d = nc.dram_tensor("r1", (ROWS, CP), dt.bfloat16, kind="ExternalInput")
    t2_d = nc.dram_tensor("t2", (ROWS, B), dt.bfloat16, kind="ExternalInput")
    out_d = nc.dram_tensor("acc_out", (128, 16), dt.float32, kind="ExternalOutput")

    NIB = ROWS // 128  # 4 row blocks
    NHALF = 2          # 2048-column halves per plane
    GCOL = 2048        # psum group width (4 banks)

    with tile.TileContext(nc) as tc:
        with (
            tc.tile_pool(name="const", bufs=1) as constp,
            tc.tile_pool(name="rhsp", bufs=1) as rhsp,
            tc.tile_pool(name="maskp", bufs=3) as maskp,
            tc.tile_pool(name="d8p", bufs=3) as d8p,
            tc.tile_pool(name="ep", bufs=2) as ep,
            tc.tile_pool(name="accp", bufs=1) as accp,
            tc.tile_pool(name="psp", bufs=2, space="PSUM") as psp,
        ):
            lhsA = constp.tile([128, ROWS], dt.bfloat16)
            nc.sync.dma_start(lhsA[:], lhs_d[0:128, :])
            lhsB = constp.tile([KDIM - 128, ROWS], dt.bfloat16)
            nc.sync.dma_start(lhsB[:], lhs_d[128:KDIM, :])

            rsA = rhsp.tile([128, CP], dt.bfloat16)
            nc.sync.dma_start(rsA[:], rhs_s_d[0:128, :])
            rsB = rhsp.tile([KDIM - 128, CP], dt.bfloat16)
            nc.sync.dma_start(rsB[:], rhs_s_d[128:KDIM, :])
            rgA = rhsp.tile([128, B], dt.bfloat16)
            nc.sync.dma_start(rgA[:], rhs_g_d[0:128, :])
            rgB = rhsp.tile([KDIM - 128, B], dt.bfloat16)
            nc.sync.dma_start(rgB[:], rhs_g_d[128:KDIM, :])

            bias_t = constp.tile([128, 1], dt.float32)
            nc.gpsimd.memset(bias_t[:], float(DELTA2))

            acc_all = accp.tile([128, 16], dt.float32)

            planes = [(rsA, rsB, r1_d), (rgA, rgB, t2_d)]
            k = 0
            for pi, (rA, rB, mask_d) in enumerate(planes):
                for ib in range(NIB):
                    mask_t = maskp.tile([128, 4096], dt.bfloat16, tag="mask")
                    nc.sync.dma_start(
                        mask_t[:], mask_d[ib * 128:(ib + 1) * 128, :]
                    )
                    lA = lhsA[:, ib * 128:(ib + 1) * 128]
                    lB = lhsB[:, ib * 128:(ib + 1) * 128]
                    for half in range(NHALF):
                        ps = psp.tile([128, GCOL], dt.float32, tag="ps")
                        for cc in range(GCOL // 512):
                            c0 = half * GCOL + cc * 512
                            o = ps[:, cc * 512:(cc + 1) * 512]
                            nc.tensor.matmul(
                                o, lA, rA[:, c0:c0 + 512], start=True, stop=False
                            )
                            nc.tensor.matmul(
                                o, lB, rB[:, c0:c0 + 512], start=False, stop=True
                            )
                        d8 = d8p.tile([128, GCOL], dt.bfloat16, tag="d8")
                        nc.scalar.activation(
                            d8[:], ps[:], mybir.ActivationFunctionType.Sqrt,
                            bias=bias_t[:], scale=64.0,
                        )
                        et = ep.tile([128, GCOL], dt.bfloat16, tag="et")
                        nc.vector.scalar_tensor_tensor(
                            out=et[:],
                            in0=d8[:],
                            scalar=1.0,
                            in1=mask_t[:, half * GCOL:(half + 1) * GCOL],
                            op0=mybir.AluOpType.mult,
                            op1=mybir.AluOpType.mult,
                            accum_out=acc_all[:, k:k + 1],
                        )
                        k += 1

            nc.sync.dma_start(out_d[:], acc_all[:])

    nc.compile()
    return nc


def _get_nc():
    if "nc" not in _CACHE:
        _CACHE["nc"] = _build_nc()
    return _CACHE["nc"]


def kernel(y_true, y_pred, lookup, global_step, current_epoch, _want_trace=False):
    from concourse.bass_utils import run_bass_kernel_spmd

    y_true = np.asarray(y_true).astype(np.int64)
    y_pred = np.asarray(y_pred, dtype=np.float32)
    lookup = np.asarray(lookup, dtype=np.float32)
    gs = int(np.asarray(global_step))

    if gs < MOM_WARMUP:
        momentum = 0.5 + (BASE_MOM - 0.5) * (gs / MOM_WARMUP)
    else:
        momentum = BASE_MOM
    progress = min(1.0, (gs - WARMUP_STEPS) / 5000.0)
    aw = MEMORY_WEIGHT * progress

    # ---- host: normalize, bank scatter-EMA, compaction ----
    z = y_pred[:, :D]
    nrm = np.sqrt((z.astype(np.float64) ** 2).sum(axis=1))
    zn = (z / np.maximum(nrm, EPS)[:, None]).astype(np.float32)

    valid = (y_true >= 0) & (y_true < N_CLASSES)
    bg = ~valid
    nv = int(valid.sum())
    lc = np.clip(y_true, 0, N_CLASSES - 1)

    bank = _bank_chains(zn, y_true, momentum)
    init_list = np.array(sorted(bank.keys()), dtype=np.int64)
    C = len(init_list)
    assert C <= CP, f"too many initialized classes: {C} > {CP}"

    zn_bf = cap_bf16(zn)
    bank_rows = (
        np.stack([bank[c] for c in init_list])
        if C else np.zeros((0, D), np.float32)
    )
    bank_bf = cap_bf16(bank_rows)

    znd = zn_bf.astype(np.float64)
    bankd = bank_bf.astype(np.float64)

    # ---- host: exact linear terms (fp64) ----
    R = lookup[lc]                    # (B, 8192)
    R_init = R[:, init_list]          # (B, C)
    A_S = 2.0 * nv * C - 2.0 * float(znd[valid].sum(0) @ bankd.sum(0))
    B_S = nv * C - float(R_init[valid].sum(dtype=np.float64))

    T_full = R[:, lc]                 # (B, B) = lookup[lc_i, lc_j]
    tu = np.triu(T_full, 1)
    tsym = tu + tu.T
    both_bg = bg[:, None] & bg[None, :]
    one_bg = bg[:, None] ^ bg[None, :]
    tsym = np.where(both_bg, np.float32(BG_SIM),
                    np.where(one_bg, np.float32(BG_OTHER_SIM), tsym))
    np.fill_diagonal(tsym, 0.0)

    Np = B * (B - 1) // 2
    szn = znd.sum(0)
    sumG_offdiag = float(szn @ szn) - float((znd ** 2).sum())
    A_G = 2.0 * Np - sumG_offdiag
    B_G = Np - float(tsym.sum(dtype=np.float64)) / 2.0

    # ---- device operand construction ----
    ones_f = np.float32(1.0)
    rhs_s = np.zeros((KDIM, CP), dtype=bf16)
    if C:
        rhs_s[0:D, 0:C] = (bank_bf.astype(np.float32).T * np.float32(-2.0)).astype(bf16)
    rhs_s[D, :] = bf16(2.0)

    rhs_g = np.zeros((KDIM, B), dtype=bf16)
    rhs_g[0:D, :] = (zn_bf.astype(np.float32).T * np.float32(-2.0)).astype(bf16)
    rhs_g[D, :] = bf16(2.0)

    T2 = (1.0 - tsym).astype(np.float32)
    np.fill_diagonal(T2, 0.0)
    T2 = T2.astype(bf16)

    R1c_full = np.zeros((B, CP), dtype=bf16)
    if C:
        R1c_full[:, 0:C] = ((1.0 - R_init) * valid[:, None]).astype(bf16)

    in_maps = []
    for c in range(NCORES):
        rows = slice(c * ROWS, (c + 1) * ROWS)
        lhs = np.empty((KDIM, ROWS), dtype=bf16)
        lhs[0:D, :] = zn_bf[rows].T
        lhs[D, :] = ones_f
        in_maps.append({
            "lhs": np.ascontiguousarray(lhs),
            "rhs_s": rhs_s,
            "rhs_g": rhs_g,
            "r1": np.ascontiguousarray(R1c_full[rows]),
            "t2": np.ascontiguousarray(T2[rows]),
        })

    nc = _get_nc()
    if _want_trace:
        import tempfile
        try:
            from trn_agent_boot.trn_boot import _ntff_profile_via_ctypes
            hook = _ntff_profile_via_ctypes("/opt/axon/libaxon_pjrt.so")
            outdir = tempfile.mkdtemp(prefix="ntff_")
            with hook(outdir, [0]):
                res = run_bass_kernel_spmd(nc, in_maps, list(range(NCORES)))
            _CACHE["last_profile_dir"] = outdir
        except Exception as e:  # profiling unavailable -> run untraced
            _CACHE["trace_error"] = repr(e)
            res = run_bass_kernel_spmd(nc, in_maps, list(range(NCORES)))
        _CACHE["last_results"] = res
    else:
        res = run_bass_kernel_spmd(nc, in_maps, list(range(NCORES)))

    P3S = 0.0
    P3G = 0.0
    for r in res.results:
        acc = np.asarray(r["acc_out"], dtype=np.float64)
        P3S += float(acc[:, 0:8].sum())
        P3G += float(acc[:, 8:16].sum())

    mem_sum = A_S + 16.0 * B_S - P3S
    denom = max(nv * C, 1)
    mem_loss = mem_sum / denom

    batch_sum = A_G + 16.0 * B_G - P3G / 2.0
    batch_loss = batch_sum / Np

    loss = (1.0 - aw) * batch_loss + aw * mem_loss
    return np.float32(loss)
